# revision 13
# baseline (speedup 1.0000x reference)
"""Block-sparse linear layer (x @ (mask*W).T + bias) on 8 TRN2 NeuronCores.

Strategy: data-parallel over batch rows (1024 rows of x per core), with a
mixed-precision K split per output tile. The latin-square block mask keeps 8
of 16 k-blocks per output block-row; of those, the 2 blocks lying in
S = {0, 4, 8, 12} (every 8-long cyclic window contains exactly 2) are
computed in fp8-e4m3 with DoubleRow matmuls (K=256 per instruction, 2x PE
throughput), the other 6 in bf16. That cuts PE time 12.5% while the fp8
quantization noise stays ~1.5e-2 absmax-relative (< 2e-2 tolerance).

All operands are pre-scaled (x by 16, W by 256; exact in bf16, keeps fp8
normal) so every PSUM partial lands in x4096 space; eviction fuses the
1/4096 descale and bias add in one vector/scalar op per half-tile.
"""

import sys
import types

import numpy as np
import ml_dtypes

BATCH = 8192
SIZE = 4096
NB = 16
BLOCK = 256
NCORES = 8
MC = BATCH // NCORES  # 1024 rows per core
P = 128
KS = SIZE // P  # 32 k-subtiles
OT = SIZE // P  # 32 o-tiles
HALF = 512
XS = 16.0
WS = 256.0
FP8_MAX = 240.0

S_BLOCKS = (0, 4, 8, 12)
X8_SLOT_BLOCKS = (0, 4, 8, 12, 0)  # block 0 duplicated so pairs are adjacent
N_WARM = 10

_BUILD_CACHE = {}


def _install_ntff_hook():
    if "antenv.axon_hooks" in sys.modules:
        return
    try:
        from trn_agent_boot.trn_boot import _ntff_profile_via_ctypes

        hook = _ntff_profile_via_ctypes("/opt/axon/libaxon_pjrt.so")
        mod = types.ModuleType("antenv.axon_hooks")
        mod.get_axon_ntff_profile_hook = lambda: hook
        sys.modules["antenv.axon_hooks"] = mod
    except Exception:
        pass


def _keep(i, j):
    return (i + j) % NB >= NB // 2


def _pair_slot(i):
    # fp8 pair of block-row i is (slot k, slot k+1) in X8_SLOT_BLOCKS
    return [2, 1, 0, 3][i // 4]


def _bf16_blocks(i):
    """bf16 blocks of row i, in global first-use order."""
    pair = {X8_SLOT_BLOCKS[_pair_slot(i)], X8_SLOT_BLOCKS[_pair_slot(i) + 1]}
    blocks = [j for j in range(NB) if _keep(i, j) and j not in pair]
    return sorted(blocks, key=_FO.index)


def _first_use_order():
    fo = []
    for i in range(NB):
        pair = {X8_SLOT_BLOCKS[_pair_slot(i)], X8_SLOT_BLOCKS[_pair_slot(i) + 1]}
        for j in range(NB):
            if _keep(i, j) and j not in pair and j not in fo:
                fo.append(j)
    return fo


_FO = _first_use_order()  # 12 bf16 blocks in first-use order
NXB = 2 * len(_FO)  # 24 bf16 x slabs


def _build():
    import concourse.mybir as mybir
    import concourse.tile as tile
    from concourse import bacc

    bf16, f32, f8 = mybir.dt.bfloat16, mybir.dt.float32, mybir.dt.float8e4
    DR = mybir.MatmulPerfMode.DoubleRow
    nc = bacc.Bacc("TRN2", target_bir_lowering=False)

    xb_d = nc.declare_dram_parameter("xb", [P, NXB, MC], bf16, isOutput=False)
    x8_d = nc.declare_dram_parameter("x8", [P, 10, MC], f8, isOutput=False)
    # wb for tiles 0..3 packed as one [P, 48, P] tensor (12KB rows -> few
    # DMA descriptors on the startup-critical path); the rest stream per-tile
    wbq_d = nc.declare_dram_parameter("wbq", [P, 48, P], bf16, isOutput=False)
    wb_d = nc.declare_dram_parameter("wb", [OT, P, 12, P], bf16, isOutput=False)
    w8_d = nc.declare_dram_parameter("w8", [P, OT * 4, P], f8, isOutput=False)
    bias_d = nc.declare_dram_parameter("biast", [P, OT], f32, isOutput=False)
    out_d = nc.declare_dram_parameter("out", [OT, P, MC], bf16, isOutput=True)

    with tile.TileContext(nc) as tc:
        with (
            tc.tile_pool(name="const", bufs=1) as const_pool,
            tc.tile_pool(name="xbp", bufs=1) as xbp,
            tc.tile_pool(name="x8p", bufs=1) as x8p,
            tc.tile_pool(name="w8p", bufs=1) as w8p,
            tc.tile_pool(name="wbp", bufs=8) as wbp,
            tc.tile_pool(name="opool", bufs=4) as opool,
            tc.tile_pool(name="psum", bufs=4, space="PSUM") as psum_pool,
        ):
            # Warm the PE clock (HAM un-throttles after ~3.4us of sustained
            # gapless matmul activity) while the first DMAs are in flight.
            warm = const_pool.tile([P, HALF], bf16, name="warm")
            nc.gpsimd.memset(warm[:], 0)
            warm_ps = psum_pool.tile([P, HALF], f32, name="warm_ps", tag="ps")
            for i in range(N_WARM):
                nc.tensor.matmul(
                    warm_ps[:],
                    lhsT=warm[:, 0:P],
                    rhs=warm[:],
                    start=(i == 0),
                    stop=(i == N_WARM - 1),
                )

            bias_tile = const_pool.tile([P, OT], f32)
            xb_t = xbp.tile([P, NXB, MC], bf16)
            x8_t = x8p.tile([P, 10, MC], f8)
            w8_t = w8p.tile([P, OT * 4, P], f8)
            wbq_t = const_pool.tile([P, 48, P], bf16, name="wbq")
            wb_tiles = {}

            def wb_ap(t, sidx):
                if t < 4:
                    return wbq_t[:, t * 12 + sidx, :]
                return wb_tiles[t][:, sidx, :]

            def wb_dma(t, engine):
                wb_tiles[t] = wbp.tile([P, 12, P], bf16, name="wb")
                engine.dma_start(out=wb_tiles[t][:], in_=wb_d[t])

            def x8_dma(slots, engine, u):
                lo, hi = slots
                engine.dma_start(
                    out=x8_t[:, u * 5 + lo : u * 5 + hi, :],
                    in_=x8_d[:, u * 5 + lo : u * 5 + hi, :],
                )

            def xb_dma(b, engine):
                fi = _FO.index(b)
                engine.dma_start(
                    out=xb_t[:, 2 * fi : 2 * fi + 2, :],
                    in_=xb_d[:, 2 * fi : 2 * fi + 2, :],
                )

            # Startup-critical loads use few, large-row DMA starts (the ring
            # processes ~one per-partition descriptor row per 15ns, so many
            # small starts serialize the startup). Consumption order: all of
            # x8 (one 10KB-row start), w8 for tiles 0..3, the packed wb quad
            # (12KB rows), then the row-0/1 bf16 x slabs in 3-block groups.
            nc.sync.dma_start(out=x8_t[:], in_=x8_d[:])
            nc.sync.dma_start(out=w8_t[:, 0:16, :], in_=w8_d[:, 0:16, :])
            nc.sync.dma_start(out=wbq_t[:], in_=wbq_d[:])
            nc.sync.dma_start(out=xb_t[:, 0:6, :], in_=xb_d[:, 0:6, :])
            nc.gpsimd.dma_start(out=w8_t[:, 16:, :], in_=w8_d[:, 16:, :])
            nc.sync.dma_start(out=xb_t[:, 12:14, :], in_=xb_d[:, 12:14, :])
            nc.sync.dma_start(out=xb_t[:, 6:12, :], in_=xb_d[:, 6:12, :])
            wb_dma(4, nc.gpsimd)
            wb_dma(5, nc.gpsimd)
            nc.gpsimd.dma_start(out=bias_tile[:], in_=bias_d[:])
            nc.sync.dma_start(out=xb_t[:, 14:18, :], in_=xb_d[:, 14:18, :])
            wb_dma(6, nc.gpsimd)
            wb_dma(7, nc.gpsimd)
            nc.sync.dma_start(out=xb_t[:, 18:24, :], in_=xb_d[:, 18:24, :])

            ps = {}
            n_mm = {}

            def start_tile(t):
                ps[t] = psum_pool.tile([P, MC], f32, name="ps", tag="ps")
                n_mm[t] = [0, 0]

            def mm(t, h, lhsT, rhs, pm=None):
                n_mm[t][h] += 1
                nc.tensor.matmul(
                    ps[t][:, h * HALF : (h + 1) * HALF],
                    lhsT=lhsT,
                    rhs=rhs,
                    start=(n_mm[t][h] == 1),
                    stop=(n_mm[t][h] == 14),
                    perf_mode=pm,
                )

            def dr(t):
                k = _pair_slot(t // 2)
                for u in (0, 1):
                    for h in (0, 1):
                        mm(
                            t,
                            h,
                            w8_t[:, t * 4 + u * 2 : t * 4 + u * 2 + 2, :],
                            x8_t[:, u * 5 + k : u * 5 + k + 2, h * HALF : (h + 1) * HALF],
                            pm=DR,
                        )

            def bf(t, b, u):
                i = t // 2
                sidx = 2 * _bf16_blocks(i).index(b) + u
                fi = _FO.index(b)
                for h in (0, 1):
                    mm(
                        t,
                        h,
                        wb_ap(t, sidx),
                        xb_t[:, 2 * fi + u, h * HALF : (h + 1) * HALF],
                    )

            def evict(t, quarters=False):
                # Halves (or quarters for the final tiles) alternate between
                # the Vector and Scalar engines so they run in parallel; the
                # out-DMA triggers go on the sync/gpsimd rings (a DIRECT2D
                # trigger costs ~590ns and must not serialize the evictions).
                o = opool.tile([P, MC], bf16, name="o_tile")
                n = 4 if quarters else 2
                step = MC // n
                for q in range(n):
                    sl = slice(q * step, (q + 1) * step)
                    if (t + q) % 2 == 0:
                        nc.vector.tensor_scalar(
                            o[:, sl],
                            ps[t][:, sl],
                            1.0 / (XS * WS),
                            bias_tile[:, t : t + 1],
                            op0=mybir.AluOpType.mult,
                            op1=mybir.AluOpType.add,
                        )
                    else:
                        nc.scalar.activation(
                            o[:, sl],
                            ps[t][:, sl],
                            mybir.ActivationFunctionType.Identity,
                            bias=bias_tile[:, t : t + 1],
                            scale=1.0 / (XS * WS),
                        )
                nc.sync.dma_start(out=out_d[t], in_=o[:])

            # Rows 0+1 as one 4-tile group: chunk-major over the union of
            # their bf16 blocks maximizes PE work per arriving x slab.
            quad = (0, 1, 2, 3)
            for t in quad:
                start_tile(t)
            for t in quad:
                dr(t)
            union = []
            for b in _bf16_blocks(0) + _bf16_blocks(1):
                if b not in union:
                    union.append(b)
            for b in sorted(union, key=_FO.index):
                for t in quad:
                    if b in _bf16_blocks(t // 2):
                        for u in (0, 1):
                            bf(t, b, u)
            for t in quad:
                evict(t)

            for m in range(2, NB):
                t0, t1 = 2 * m, 2 * m + 1
                if m + 2 < NB:  # prefetch wb two rows ahead
                    wb_dma(t0 + 4, nc.gpsimd)
                    wb_dma(t1 + 4, nc.gpsimd)
                start_tile(t0)
                start_tile(t1)
                last = m == NB - 1
                if last:
                    # tile-major so t30's eviction overlaps t31's matmuls
                    dr(t0)
                    for b in _bf16_blocks(m):
                        for u in (0, 1):
                            bf(t0, b, u)
                    evict(t0, quarters=True)
                    dr(t1)
                    for b in _bf16_blocks(m):
                        for u in (0, 1):
                            bf(t1, b, u)
                    evict(t1, quarters=True)
                else:
                    dr(t0)
                    dr(t1)
                    for b in _bf16_blocks(m):
                        for t in (t0, t1):
                            for u in (0, 1):
                                bf(t, b, u)
                    evict(t0)
                    evict(t1)
    nc.compile()
    return nc


def _get_kernel():
    if "nc" not in _BUILD_CACHE:
        _BUILD_CACHE["nc"] = _build()
    return _BUILD_CACHE["nc"]


def _expected_mask(mask):
    m4 = np.asarray(mask).reshape(NB, BLOCK, NB, BLOCK)
    keep = m4[:, 0, :, 0]
    if not np.all(m4 == keep[:, None, :, None]):
        return False
    i = np.arange(NB)
    return np.array_equal(keep, ((i[:, None] + i[None, :]) % NB) >= NB // 2)


def _to_fp8(a):
    return np.clip(a, -FP8_MAX, FP8_MAX).astype(ml_dtypes.float8_e4m3)


def kernel(x, weight, bias, mask, _trace=False):
    from concourse.bass_utils import run_bass_kernel_spmd

    _install_ntff_hook()

    x = np.asarray(x)
    weight = np.asarray(weight)
    bias = np.asarray(bias, dtype=np.float32)
    if not _expected_mask(mask):
        w = np.where(np.asarray(mask), weight, 0.0).astype(np.float32)
        out = x.astype(np.float32) @ w.T + bias
        return (out, None) if _trace else out

    nc = _get_kernel()

    ws = (weight * WS).astype(np.float32)  # [out, k]

    # wb[t, p, s, f] = ws[t*P+f, ks(s)*P + p] for the 12 bf16 subtiles of t
    wb = np.empty((OT, P, 12, P), dtype=ml_dtypes.bfloat16)
    # w8[p, t*4 + u*2 + i, f] = ws[t*P+f, blk(k+i)*BLOCK + u*P + p]
    w8 = np.empty((P, OT * 4, P), dtype=ml_dtypes.float8_e4m3)
    for t in range(OT):
        i_row = t // 2
        wt = ws[t * P : (t + 1) * P].reshape(P, KS, P)  # [f, ks, p]
        subs = [2 * b + u for b in _bf16_blocks(i_row) for u in (0, 1)]
        wb[t] = wt[:, subs, :].transpose(2, 1, 0).astype(ml_dtypes.bfloat16)
        k = _pair_slot(i_row)
        for u in (0, 1):
            for i in (0, 1):
                blk = X8_SLOT_BLOCKS[k + i]
                w8[:, t * 4 + u * 2 + i, :] = _to_fp8(
                    wt[:, 2 * blk + u, :].T
                )

    biast = np.ascontiguousarray(bias.reshape(OT, P).T, dtype=np.float32)
    wbq = np.ascontiguousarray(wb[0:4].transpose(1, 0, 2, 3)).reshape(P, 48, P)

    in_maps = []
    for c in range(NCORES):
        xc = x[c * MC : (c + 1) * MC, :].astype(np.float32) * XS  # [MC, SIZE]
        xt = xc.reshape(MC, KS, P).transpose(2, 1, 0)  # [P, KS, MC]
        xb_subs = [2 * b + u for b in _FO for u in (0, 1)]
        xb = np.ascontiguousarray(xt[:, xb_subs, :]).astype(ml_dtypes.bfloat16)
        x8_subs = [2 * b + u for u in (0, 1) for b in X8_SLOT_BLOCKS]
        x8 = _to_fp8(np.ascontiguousarray(xt[:, x8_subs, :]))
        in_maps.append(
            {"xb": xb, "x8": x8, "wbq": wbq, "wb": wb, "w8": w8, "biast": biast}
        )

    res = run_bass_kernel_spmd(nc, in_maps, list(range(NCORES)), trace=_trace)

    out = np.empty((BATCH, SIZE), dtype=np.float32)
    for c in range(NCORES):
        o = res.results[c]["out"]  # [OT, P, MC] bf16
        out[c * MC : (c + 1) * MC, :] = o.reshape(SIZE, MC).T.astype(np.float32)
    if _trace:
        return out, res
    return out


# revision 14
# speedup vs baseline: 1.0221x; 1.0221x over previous
"""Block-sparse linear layer (x @ (mask*W).T + bias) on 8 TRN2 NeuronCores.

Strategy: data-parallel over batch rows (1024 rows of x per core), with a
mixed-precision K split per output tile. The latin-square block mask keeps 8
of 16 k-blocks per output block-row; of those, the 2 blocks lying in
S = {0, 4, 8, 12} (every 8-long cyclic window contains exactly 2) are
computed in fp8-e4m3 with DoubleRow matmuls (K=256 per instruction, 2x PE
throughput), the other 6 in bf16. That cuts PE time 12.5% while the fp8
quantization noise stays ~1.6e-2 absmax-relative (< 2e-2 tolerance).

All operands are pre-scaled (x by 16, W by 256; exact in bf16, keeps fp8
normal) so every PSUM partial lands in x4096 space; eviction fuses the
1/4096 descale and bias add in one vector/scalar op per half-tile and
writes bf16 (halves the output DMA; the host converts back to fp32).

DMA: the ring processes one per-partition descriptor row per ~15ns and a
start's semaphore fires only when the whole transfer lands, so the
startup-critical operands travel in few, fat-row starts: one 6KB-row fp8
tensor (x8 pair (8,12) + w8 tiles 0-3) feeds the first DoubleRow matmuls
right as the PE warmup ends, then a 16KB-row bf16 tensor (wb tiles 0-3 +
x block 9) feeds the first bf16 chunks.
"""

import sys
import types

import numpy as np
import ml_dtypes

BATCH = 8192
SIZE = 4096
NB = 16
BLOCK = 256
NCORES = 8
MC = BATCH // NCORES  # 1024 rows per core
P = 128
KS = SIZE // P  # 32 k-subtiles
OT = SIZE // P  # 32 o-tiles
HALF = 512
XS = 16.0
WS = 256.0
FP8_MAX = 240.0
N_WARM = 13

# fp8 blocks per block-row i: the two kept blocks in {0,4,8,12}.
# i//4 = 0: (8,12), 1: (4,8), 2: (0,4), 3: (12,0) -- sub-row order as listed.
PAIR_BLOCKS = ((8, 12), (4, 8), (0, 4), (12, 0))
# x8 slot layout: pair (8,12) lives in the startup tensor; the rest tensor
# holds slots [12, 0, 4, 8] so each later pair is an adjacent row-pair.
X8R_SLOTS = (12, 0, 4, 8)
X8R_LO = {1: 2, 2: 1, 3: 0}  # i//4 -> first row of its pair in x8r

_BUILD_CACHE = {}


def _install_ntff_hook():
    if "antenv.axon_hooks" in sys.modules:
        return
    try:
        from trn_agent_boot.trn_boot import _ntff_profile_via_ctypes

        hook = _ntff_profile_via_ctypes("/opt/axon/libaxon_pjrt.so")
        mod = types.ModuleType("antenv.axon_hooks")
        mod.get_axon_ntff_profile_hook = lambda: hook
        sys.modules["antenv.axon_hooks"] = mod
    except Exception:
        pass


def _keep(i, j):
    return (i + j) % NB >= NB // 2


def _first_use_order():
    fo = []
    for i in range(NB):
        pair = set(PAIR_BLOCKS[i // 4])
        for j in range(NB):
            if _keep(i, j) and j not in pair and j not in fo:
                fo.append(j)
    return fo


_FO = _first_use_order()  # [9,10,11,13,14,15,7,6,5,3,2,1]
_FO2 = _FO[1:]  # blocks living in xb (block 9 lives in the startB tensor)


def _bf16_blocks(i):
    pair = set(PAIR_BLOCKS[i // 4])
    blocks = [j for j in range(NB) if _keep(i, j) and j not in pair]
    return sorted(blocks, key=_FO.index)


def _build():
    import concourse.mybir as mybir
    import concourse.tile as tile
    from concourse import bacc

    bf16, f32, f8 = mybir.dt.bfloat16, mybir.dt.float32, mybir.dt.float8e4
    DR = mybir.MatmulPerfMode.DoubleRow
    nc = bacc.Bacc("TRN2", target_bir_lowering=False)

    # startA: x8 for pair (8,12) [groups: slot(2) x u(2) x half(2)] then w8
    # for tiles 0..3 [group 8+t: rows u*2+i]; 6KB rows, one DMA start.
    sa_d = nc.declare_dram_parameter("sa", [P, 12, 4, P], f8, isOutput=False)
    # startB: wb for tiles 0..3 (rows t*12+sidx) then x block 9 (rows
    # 48 + u*8 + r); 16KB rows, one DMA start.
    sb_d = nc.declare_dram_parameter("sb", [P, 64, P], bf16, isOutput=False)
    xb_d = nc.declare_dram_parameter("xbt", [P, 22, MC], bf16, isOutput=False)
    x8r_d = nc.declare_dram_parameter("x8r", [P, 4, 2, MC], f8, isOutput=False)
    w8r_d = nc.declare_dram_parameter("w8r", [P, 112, P], f8, isOutput=False)
    wb_d = nc.declare_dram_parameter("wb", [OT, P, 12, P], bf16, isOutput=False)
    bias_d = nc.declare_dram_parameter("biast", [P, OT], f32, isOutput=False)
    out_d = nc.declare_dram_parameter("out", [OT, P, MC], bf16, isOutput=True)

    with tile.TileContext(nc) as tc:
        with (
            tc.tile_pool(name="const", bufs=1) as const_pool,
            tc.tile_pool(name="xbp", bufs=1) as xbp,
            tc.tile_pool(name="wbp", bufs=8) as wbp,
            tc.tile_pool(name="opool", bufs=4) as opool,
            tc.tile_pool(name="psum", bufs=4, space="PSUM") as psum_pool,
        ):
            # Warm the PE clock (HAM un-throttles after ~3.4us of sustained
            # gapless matmul activity) while the first DMAs are in flight.
            warm = const_pool.tile([P, HALF], bf16, name="warm")
            nc.gpsimd.memset(warm[:], 0)
            warm_ps = psum_pool.tile([P, HALF], f32, name="warm_ps", tag="ps")
            for i in range(N_WARM):
                nc.tensor.matmul(
                    warm_ps[:],
                    lhsT=warm[:, 0:P],
                    rhs=warm[:],
                    start=(i == 0),
                    stop=(i == N_WARM - 1),
                )

            bias_tile = const_pool.tile([P, OT], f32)
            sa_t = const_pool.tile([P, 12, 4, P], f8, name="sa")
            sb_t = const_pool.tile([P, 64, P], bf16, name="sb")
            xb_t = xbp.tile([P, 22, MC], bf16)
            x8r_t = const_pool.tile([P, 4, 2, MC], f8, name="x8r")
            w8r_t = const_pool.tile([P, 112, P], f8, name="w8r")
            wb_tiles = {}

            def wb_dma(t, engine):
                wb_tiles[t] = wbp.tile([P, 12, P], bf16, name="wb")
                engine.dma_start(out=wb_tiles[t][:], in_=wb_d[t])

            # Sync ring: startup-critical, in consumption order.
            nc.sync.dma_start(out=sa_t[:], in_=sa_d[:])
            nc.sync.dma_start(out=sb_t[:], in_=sb_d[:])
            nc.sync.dma_start(out=xb_t[:, 0:4, :], in_=xb_d[:, 0:4, :])
            nc.sync.dma_start(out=xb_t[:, 4:8, :], in_=xb_d[:, 4:8, :])
            nc.sync.dma_start(out=xb_t[:, 8:12, :], in_=xb_d[:, 8:12, :])
            nc.sync.dma_start(out=x8r_t[:], in_=x8r_d[:])
            nc.sync.dma_start(out=xb_t[:, 12:16, :], in_=xb_d[:, 12:16, :])
            nc.sync.dma_start(out=xb_t[:, 16:22, :], in_=xb_d[:, 16:22, :])
            # GpSimd ring: weight streams (kept off the first ~10us so the
            # startup tensors get the full DMA bandwidth).
            nc.gpsimd.dma_start(out=w8r_t[:, 0:16, :], in_=w8r_d[:, 0:16, :])
            wb_dma(4, nc.gpsimd)
            wb_dma(5, nc.gpsimd)
            nc.gpsimd.dma_start(out=bias_tile[:], in_=bias_d[:])
            wb_dma(6, nc.gpsimd)
            wb_dma(7, nc.gpsimd)
            nc.gpsimd.dma_start(out=w8r_t[:, 16:, :], in_=w8r_d[:, 16:, :])

            ps = {}
            n_mm = {}

            def start_tile(t):
                ps[t] = psum_pool.tile([P, MC], f32, name="ps", tag="ps")
                n_mm[t] = [0, 0]

            def mm(t, h, lhsT, rhs, pm=None):
                n_mm[t][h] += 1
                nc.tensor.matmul(
                    ps[t][:, h * HALF : (h + 1) * HALF],
                    lhsT=lhsT,
                    rhs=rhs,
                    start=(n_mm[t][h] == 1),
                    stop=(n_mm[t][h] == 14),
                    perf_mode=pm,
                )

            def dr(t):
                g = (t // 2) // 4
                for u in (0, 1):
                    if t < 8:
                        lhsT = sa_t[:, 8 + t, u * 2 : u * 2 + 2, :] if t < 4 else None
                    if t >= 4:
                        lhsT = w8r_t[:, (t - 4) * 4 + u * 2 : (t - 4) * 4 + u * 2 + 2, :]
                    for h in (0, 1):
                        if g == 0:
                            g0 = u * 2 + h
                            rhs = sa_t[:, g0 : g0 + 5 : 4, :, :]
                        else:
                            lo = X8R_LO[g]
                            rhs = x8r_t[
                                :, lo : lo + 2, u, h * HALF : (h + 1) * HALF
                            ]
                        mm(t, h, lhsT, rhs, pm=DR)

            def wb_ap(t, sidx):
                if t < 4:
                    return sb_t[:, t * 12 + sidx, :]
                return wb_tiles[t][:, sidx, :]

            def xb_rhs(b, u, h):
                if b == 9:
                    r = 48 + u * 8 + h * 4
                    return sb_t[:, r : r + 4, :]
                fi = _FO2.index(b)
                return xb_t[:, 2 * fi + u, h * HALF : (h + 1) * HALF]

            def bf(t, b, u):
                sidx = 2 * _bf16_blocks(t // 2).index(b) + u
                for h in (0, 1):
                    mm(t, h, wb_ap(t, sidx), xb_rhs(b, u, h))

            def evict(t, quarters=False, split_out=False):
                o = opool.tile([P, MC], bf16, name="o_tile")
                n = 4 if quarters else 2
                step = MC // n
                for q in range(n):
                    sl = slice(q * step, (q + 1) * step)
                    if (t + q) % 2 == 0:
                        nc.vector.tensor_scalar(
                            o[:, sl],
                            ps[t][:, sl],
                            1.0 / (XS * WS),
                            bias_tile[:, t : t + 1],
                            op0=mybir.AluOpType.mult,
                            op1=mybir.AluOpType.add,
                        )
                    else:
                        nc.scalar.activation(
                            o[:, sl],
                            ps[t][:, sl],
                            mybir.ActivationFunctionType.Identity,
                            bias=bias_tile[:, t : t + 1],
                            scale=1.0 / (XS * WS),
                        )
                if split_out:
                    nc.sync.dma_start(out=out_d[t, :, 0:HALF], in_=o[:, 0:HALF])
                    nc.gpsimd.dma_start(out=out_d[t, :, HALF:], in_=o[:, HALF:])
                else:
                    nc.sync.dma_start(out=out_d[t], in_=o[:])

            # Rows 0+1 as one 4-tile group: chunk-major over the union of
            # their bf16 blocks maximizes PE work per arriving x slab.
            quad = (0, 1, 2, 3)
            for t in quad:
                start_tile(t)
            for t in quad:
                dr(t)
            union = []
            for b in _bf16_blocks(0) + _bf16_blocks(1):
                if b not in union:
                    union.append(b)
            for b in sorted(union, key=_FO.index):
                for t in quad:
                    if b in _bf16_blocks(t // 2):
                        for u in (0, 1):
                            bf(t, b, u)
            for t in quad:
                evict(t)

            for m in range(2, NB):
                t0, t1 = 2 * m, 2 * m + 1
                if m + 2 < NB:  # prefetch wb two rows ahead
                    wb_dma(t0 + 4, nc.gpsimd)
                    wb_dma(t1 + 4, nc.gpsimd)
                start_tile(t0)
                start_tile(t1)
                last = m == NB - 1
                if last:
                    # tile-major so t30's eviction overlaps t31's matmuls
                    dr(t0)
                    for b in _bf16_blocks(m):
                        for u in (0, 1):
                            bf(t0, b, u)
                    evict(t0, quarters=True, split_out=True)
                    dr(t1)
                    for b in _bf16_blocks(m):
                        for u in (0, 1):
                            bf(t1, b, u)
                    evict(t1, quarters=True, split_out=True)
                else:
                    dr(t0)
                    dr(t1)
                    for b in _bf16_blocks(m):
                        for t in (t0, t1):
                            for u in (0, 1):
                                bf(t, b, u)
                    evict(t0)
                    evict(t1)
    nc.compile()
    return nc


def _get_kernel():
    if "nc" not in _BUILD_CACHE:
        _BUILD_CACHE["nc"] = _build()
    return _BUILD_CACHE["nc"]


def _expected_mask(mask):
    m4 = np.asarray(mask).reshape(NB, BLOCK, NB, BLOCK)
    keep = m4[:, 0, :, 0]
    if not np.all(m4 == keep[:, None, :, None]):
        return False
    i = np.arange(NB)
    return np.array_equal(keep, ((i[:, None] + i[None, :]) % NB) >= NB // 2)


def _to_fp8(a):
    return np.clip(a, -FP8_MAX, FP8_MAX).astype(ml_dtypes.float8_e4m3)


def kernel(x, weight, bias, mask, _trace=False):
    from concourse.bass_utils import run_bass_kernel_spmd

    _install_ntff_hook()

    x = np.asarray(x)
    weight = np.asarray(weight)
    bias = np.asarray(bias, dtype=np.float32)
    if not _expected_mask(mask):
        w = np.where(np.asarray(mask), weight, 0.0).astype(np.float32)
        out = x.astype(np.float32) @ w.T + bias
        return (out, None) if _trace else out

    nc = _get_kernel()

    ws = (weight * WS).astype(np.float32)  # [out, k]

    # Per-tile weight packing. wb[t, p, s, f] = ws[t*P+f, ks(s)*P + p].
    wb = np.empty((OT, P, 12, P), dtype=ml_dtypes.bfloat16)
    # w8 flat rows: [t*4 + u*2 + i] = ws[t*P+f, blk_i*BLOCK + u*P + p]
    w8 = np.empty((P, OT * 4, P), dtype=ml_dtypes.float8_e4m3)
    for t in range(OT):
        i_row = t // 2
        wt = ws[t * P : (t + 1) * P].reshape(P, KS, P)  # [f, ks, p]
        subs = [2 * b + u for b in _bf16_blocks(i_row) for u in (0, 1)]
        wb[t] = wt[:, subs, :].transpose(2, 1, 0).astype(ml_dtypes.bfloat16)
        for u in (0, 1):
            for i in (0, 1):
                blk = PAIR_BLOCKS[i_row // 4][i]
                w8[:, t * 4 + u * 2 + i, :] = _to_fp8(wt[:, 2 * blk + u, :].T)

    biast = np.ascontiguousarray(bias.reshape(OT, P).T, dtype=np.float32)
    w8r = np.ascontiguousarray(w8[:, 16:, :])

    in_maps = []
    for c in range(NCORES):
        xc = x[c * MC : (c + 1) * MC, :].astype(np.float32) * XS  # [MC, SIZE]
        xt = xc.reshape(MC, KS, P).transpose(2, 1, 0)  # [P, KS, MC]

        # startA: [P, 12, 4, P] fp8: groups 0..7 x8 of blocks 8/12
        # (g = s*4 + u*2 + h), groups 8..11 = w8 tiles 0..3.
        sa = np.empty((P, 12, 4, P), dtype=ml_dtypes.float8_e4m3)
        for s, blk in enumerate((8, 12)):
            for u in (0, 1):
                x_sub = _to_fp8(xt[:, 2 * blk + u, :])  # [P, MC]
                sa[:, s * 4 + u * 2 : s * 4 + u * 2 + 2, :, :] = x_sub.reshape(
                    P, 2, 4, P
                )
        sa[:, 8:12, :, :] = (
            w8[:, 0:16, :].reshape(P, 4, 4, P)
        )

        # startB: [P, 64, P] bf16: rows 0..47 wb tiles 0..3, rows 48..63
        # x block 9 (u-major).
        sb = np.empty((P, 64, P), dtype=ml_dtypes.bfloat16)
        sb[:, 0:48, :] = wb[0:4].transpose(1, 0, 2, 3).reshape(P, 48, P)
        for u in (0, 1):
            sb[:, 48 + u * 8 : 56 + u * 8, :] = (
                xt[:, 2 * 9 + u, :].astype(ml_dtypes.bfloat16).reshape(P, 8, P)
            )

        xb_subs = [2 * b + u for b in _FO2 for u in (0, 1)]
        xbt = np.ascontiguousarray(xt[:, xb_subs, :]).astype(ml_dtypes.bfloat16)

        # x8r: [P, 4, 2, MC] slots [12, 0, 4, 8], u inner.
        x8r = np.empty((P, 4, 2, MC), dtype=ml_dtypes.float8_e4m3)
        for s, blk in enumerate(X8R_SLOTS):
            for u in (0, 1):
                x8r[:, s, u, :] = _to_fp8(xt[:, 2 * blk + u, :])

        in_maps.append(
            {
                "sa": sa,
                "sb": sb,
                "xbt": xbt,
                "x8r": x8r,
                "w8r": w8r,
                "wb": wb,
                "biast": biast,
            }
        )

    res = run_bass_kernel_spmd(nc, in_maps, list(range(NCORES)), trace=_trace)

    out = np.empty((BATCH, SIZE), dtype=np.float32)
    for c in range(NCORES):
        o = res.results[c]["out"]  # [OT, P, MC] bf16
        out[c * MC : (c + 1) * MC, :] = o.reshape(SIZE, MC).T.astype(np.float32)
    if _trace:
        return out, res
    return out
